# revision 16
# baseline (speedup 1.0000x reference)
"""DRAW (nn_DRAW_17497696763951) Trainium2 Bass kernel.

Data-parallel over batch: B=4096 split across 8 NeuronCores (512 rows each),
weights replicated, T=10 recurrence local per core.

Device layout is feature-major (transposed): activations live as
[feature_partitions, batch_free] so every matmul is
  out[out_feat_tile, batch] = W_tile.T @ act_tile
with weights stationary and batch (512) as the moving free dim.

Algebra used (all precomputed on host):
  att = [x, x - sigmoid(canvas), h_dec]
  gates_enc = W1@x + W2@(x - sigmoid(canvas)) + W3@h_dec + Whh@h_enc + b
            = base + W2@sigmoid(-canvas) + W3@h_dec + Whh@h_enc
  where base = (W1+W2)@x + (b_ih + b_hh - rowsum(W2)),
  using sigmoid(-c) = 1 - sigmoid(c).
  exp(y) = 1/(1/sigmoid(y) - 1)  (keeps everything in one ACT table set)

All matmul operands are fp16 (measured ~3e-4 rel err/matmul; end-to-end
mimic ~5e-4); accumulation fp32 in PSUM; canvas/c-states/kl fp32.
"""
import numpy as np

import concourse.bacc as bacc
import concourse.tile as tile
import concourse.mybir as mybir
from concourse.bass_utils import run_bass_kernel_spmd

B, X, H, Z, T = 4096, 1024, 256, 10, 10
NCORES = 8
NB = B // NCORES       # 512 batch rows per core
KX = X // 128          # 8 contraction tiles for X-dim
KH = H // 128          # 2 contraction tiles for H-dim
MT = (4 * H) // 128    # 8 gate out tiles
MX = X // 128          # 8 X out tiles

F32 = mybir.dt.float32
F16 = mybir.dt.float16
AF = mybir.ActivationFunctionType
OP = mybir.AluOpType
SIG = AF.Sigmoid
TANH = AF.Tanh
COPY = AF.Copy

GATE_FUNC = [SIG, SIG, SIG, SIG, TANH, TANH, SIG, SIG]  # i,i,f,f,g,g,o,o

_NC = None
LAST_RESULT = None


def _build():
    nc = bacc.Bacc("TRN2", target_bir_lowering=False, debug=False)

    def din(name, shape, dt=F16):
        return nc.dram_tensor(name, shape, dt, kind="ExternalInput")

    xT_d = din("xT", (128, KX, NB))
    eps_d = din("epsT", (T, Z, NB), F32)
    Wsum_d = din("WsumT", (128, KX, MT, 128))
    W2_d = din("W2T", (128, KX, MT, 128))
    W3_d = din("W3T", (128, KH, MT, 128))
    WhhE_d = din("WhhET", (128, KH, MT, 128))
    WhhD_d = din("WhhDT", (128, KH, MT, 128))
    Wwr_d = din("WwrT", (128, KH, MX, 128))
    Wvar_d = din("WvarT", (128, KH, 2 * Z))
    Wihd_d = din("WihdT", (Z, MT, 128))
    Wobs_d = din("WobsT", (128, KX, MX, 128))
    id_d = din("ident", (128, 128))
    be_d = din("benc", (128, MT), F32)
    be0_d = din("benc0", (128, MT), F32)
    bd_d = din("bdec", (128, MT), F32)
    bo_d = din("bobs", (128, MX), F32)
    xmu_d = nc.dram_tensor("xmuT", (128, MX, NB), F32, kind="ExternalOutput")
    kl_d = nc.dram_tensor("klT", (Z, NB), F32, kind="ExternalOutput")

    with tile.TileContext(nc) as tc:
        with (
            tc.tile_pool(name="p1", bufs=1) as p1,
            tc.tile_pool(name="p2", bufs=2) as p2,
            tc.tile_pool(name="p3", bufs=3) as p3,
            tc.tile_pool(name="pg", bufs=12) as pg,
            tc.tile_pool(name="ps", bufs=8, space="PSUM") as psp,
        ):
            # ---- persistent weights ----
            tWsum = p1.tile([128, KX, MT, 128], F16, tag="wbig")  # later reused by Wobs
            tx = p1.tile([128, KX, NB], F16, tag="xbig")          # later reused by xmu
            tW2 = p1.tile([128, KX, MT, 128], F16, tag="w2")
            tW3 = p1.tile([128, KH, MT, 128], F16, tag="w3")
            tWhhE = p1.tile([128, KH, MT, 128], F16, tag="whhe")
            tWhhD = p1.tile([128, KH, MT, 128], F16, tag="whhd")
            tWwr = p1.tile([128, KH, MX, 128], F16, tag="wwr")
            tWvar = p1.tile([128, KH, 2 * Z], F16, tag="wvar")
            tWihd = p1.tile([Z, MT, 128], F16, tag="wihd")
            tbe = p1.tile([128, MT], F32, tag="be")
            tbe0 = p1.tile([128, MT], F32, tag="be0")
            tbd = p1.tile([128, MT], F32, tag="bd")
            tbo = p1.tile([128, MX], F32, tag="bo")
            tident = p1.tile([128, 128], F16, tag="ident")
            tbase = p1.tile([128, MT, NB], F16, tag="base")
            tcanvas = p1.tile([128, MX, NB], F32, tag="canvas")
            tkl = p1.tile([Z, NB], F32, tag="kl")

            # load order: base-critical, then step0-critical, then the rest
            nc.sync.dma_start(tx[:], xT_d[:])
            nc.sync.dma_start(tident[:], id_d[:])
            nc.sync.dma_start(tWsum[:], Wsum_d[:])
            nc.sync.dma_start(tbe[:], be_d[:])
            nc.sync.dma_start(tbe0[:], be0_d[:])
            nc.sync.dma_start(tWvar[:], Wvar_d[:])
            nc.sync.dma_start(tWihd[:], Wihd_d[:])
            nc.sync.dma_start(tbd[:], bd_d[:])
            nc.sync.dma_start(tWwr[:], Wwr_d[:])
            nc.sync.dma_start(tWhhD[:], WhhD_d[:])
            nc.sync.dma_start(tWhhE[:], WhhE_d[:])
            nc.sync.dma_start(tW3[:], W3_d[:])
            nc.sync.dma_start(tW2[:], W2_d[:])

            # ---- base = (W1+W2) @ x  (+ enc bias via ACT) ----
            base_ps = []
            for m in range(MT):
                ps = psp.tile([128, NB], F32, tag="ps")
                for k in range(KX):
                    nc.tensor.matmul(ps[:], tWsum[:, k, m, :], tx[:, k, :],
                                     start=(k == 0), stop=(k == KX - 1))
                nc.vector.tensor_scalar_add(tbase[:, m, :], ps[:],
                                            tbe[:, m:m + 1])
                base_ps.append(ps)

            def lstm_tail(gact, c_new, c_old, h_new):
                """c_new = sig(f)*c_old + sig(i)*tanh(g); h_new = sig(o)*tanh(c)"""
                for j in range(KH):
                    tmp = pg.tile([128, NB], F32, tag="ga")
                    nc.vector.tensor_mul(tmp[:], gact[j][:], gact[4 + j][:])
                    if c_old is None:
                        nc.vector.tensor_copy(c_new[:, j, :], tmp[:])
                    else:
                        nc.vector.tensor_mul(c_new[:, j, :], gact[2 + j][:],
                                             c_old[:, j, :])
                        nc.vector.tensor_add(c_new[:, j, :], c_new[:, j, :], tmp[:])
                    tct = pg.tile([128, NB], F32, tag="ga")
                    nc.scalar.activation(tct[:], c_new[:, j, :], TANH)
                    nc.vector.tensor_mul(h_new[:, j, :], gact[6 + j][:], tct[:])

            he = hd = ce = cd = None
            sprime = None

            for t in range(T):
                # ===== encoder gates =====
                gact = []
                if t == 0:
                    for m in range(MT):
                        g = pg.tile([128, NB], F32, tag="ga")
                        nc.scalar.activation(g[:], base_ps[m][:], GATE_FUNC[m],
                                             bias=tbe0[:, m:m + 1])
                        gact.append(g)
                else:
                    # pass 1: base (via identity matmul) + h-term matmuls for
                    # all 8 banks first (none depend on s'), then the W2 k-loop.
                    ps_list = [psp.tile([128, NB], F32, tag="ps",
                                        name=f"encps{t}_{m}")
                               for m in range(MT)]
                    for m in range(MT):
                        ps = ps_list[m]
                        nc.tensor.matmul(ps[:], tident[:], tbase[:, m, :],
                                         start=True, stop=False,
                                         skip_group_check=True)
                        nc.tensor.matmul(ps[:], tW3[:, 0, m, :], hd[:, 0, :],
                                         start=False, stop=False,
                                         skip_group_check=True)
                        nc.tensor.matmul(ps[:], tW3[:, 1, m, :], hd[:, 1, :],
                                         start=False, stop=False,
                                         skip_group_check=True)
                        nc.tensor.matmul(ps[:], tWhhE[:, 0, m, :], he[:, 0, :],
                                         start=False, stop=False,
                                         skip_group_check=True)
                        nc.tensor.matmul(ps[:], tWhhE[:, 1, m, :], he[:, 1, :],
                                         start=False, stop=False,
                                         skip_group_check=True)
                    for m in range(MT):
                        ps = ps_list[m]
                        for k in range(KX):
                            nc.tensor.matmul(ps[:], tW2[:, k, m, :], sprime[:, k, :],
                                             start=False, stop=(k == KX - 1),
                                             skip_group_check=True)

                # decoder psum + hh-matmuls for m0-3 fill the PE gap while
                # the encoder LSTM tail (ACT/DVE) runs
                ps_dec = [psp.tile([128, NB], F32, tag="ps",
                                   name=f"decps{t}_{m}")
                          for m in range(MT)]
                if t > 0:
                    for m in range(4):
                        ps = ps_dec[m]
                        nc.tensor.matmul(ps[:], tWhhD[:, 0, m, :], hd[:, 0, :],
                                         start=True, stop=False,
                                         skip_group_check=True)
                        nc.tensor.matmul(ps[:], tWhhD[:, 1, m, :], hd[:, 1, :],
                                         start=False, stop=False,
                                         skip_group_check=True)
                    for m in range(MT):
                        g = pg.tile([128, NB], F32, tag="ga")
                        nc.scalar.activation(g[:], ps_list[m][:], GATE_FUNC[m])
                        gact.append(g)

                ce_new = p2.tile([128, KH, NB], F32, tag="ce")
                he_new = p2.tile([128, KH, NB], F16, tag="he")
                lstm_tail(gact, ce_new, ce, he_new)
                ce, he = ce_new, he_new

                # ===== variational head: q = Wvar @ h_enc ===== (split mu/log)
                ps_mu = psp.tile([Z, NB], F32, tag="ps")
                ps_lg = psp.tile([Z, NB], F32, tag="ps")
                for k in range(KH):
                    nc.tensor.matmul(ps_mu[:], tWvar[:, k, 0:Z], he[:, k, :],
                                     start=(k == 0), stop=(k == KH - 1))
                for k in range(KH):
                    nc.tensor.matmul(ps_lg[:], tWvar[:, k, Z:2 * Z], he[:, k, :],
                                     start=(k == 0), stop=(k == KH - 1))

                # remaining decoder hh-matmuls fill the PE gap during the
                # q_std/z pointwise chain
                if t > 0:
                    for m in range(4, MT):
                        ps = ps_dec[m]
                        nc.tensor.matmul(ps[:], tWhhD[:, 0, m, :], hd[:, 0, :],
                                         start=True, stop=False,
                                         skip_group_check=True)
                        nc.tensor.matmul(ps[:], tWhhD[:, 1, m, :], hd[:, 1, :],
                                         start=False, stop=False,
                                         skip_group_check=True)

                # q_std = exp(q_log) = sigmoid(q_log)/sigmoid(-q_log)
                s = p3.tile([Z, NB], F32, tag="qs")
                nc.scalar.activation(s[:], ps_lg[:], SIG)
                sm = p3.tile([Z, NB], F32, tag="qr")
                nc.scalar.activation(sm[:], ps_lg[:], SIG, scale=-1.0)
                r = p3.tile([Z, NB], F32, tag="qr")
                nc.vector.reciprocal_approx_fast(r[:], sm[:])
                qstd = p3.tile([Z, NB], F32, tag="qs")
                nc.vector.tensor_mul(qstd[:], s[:], r[:])

                teps = p2.tile([Z, NB], F32, tag="eps")
                nc.sync.dma_start(teps[:], eps_d[t])
                w = p3.tile([Z, NB], F32, tag="qr")
                nc.vector.tensor_mul(w[:], qstd[:], teps[:])
                zt = p2.tile([Z, NB], F16, tag="z")
                nc.vector.tensor_add(zt[:], w[:], ps_mu[:])

                # kl += -q_log + 0.5*(q_std^2 + q_mu^2) - 0.5
                a = p3.tile([Z, NB], F32, tag="qs")
                nc.vector.tensor_mul(a[:], qstd[:], qstd[:])
                b2 = p3.tile([Z, NB], F32, tag="qr")
                nc.scalar.activation(b2[:], ps_mu[:], AF.Square)
                nc.vector.tensor_add(a[:], a[:], b2[:])
                # a = 0.5*a - q_log
                nc.vector.scalar_tensor_tensor(a[:], a[:], 0.5, ps_lg[:],
                                               op0=OP.mult, op1=OP.subtract)
                if t == 0:
                    nc.vector.tensor_scalar_add(tkl[:], a[:], -0.5)
                else:
                    nc.vector.scalar_tensor_tensor(tkl[:], a[:], 0.5, tkl[:],
                                                   op0=OP.subtract, op1=OP.add)

                # ===== decoder gates ===== (z matmul closes each bank)
                gact = []
                for m in range(MT):
                    ps = ps_dec[m]
                    nc.tensor.matmul(ps[:], tWihd[:, m, :], zt[:],
                                     start=(t == 0), stop=True,
                                     skip_group_check=True)
                    g = pg.tile([128, NB], F32, tag="ga")
                    nc.scalar.activation(g[:], ps[:], GATE_FUNC[m],
                                         bias=tbd[:, m:m + 1])
                    gact.append(g)

                cd_new = p2.tile([128, KH, NB], F32, tag="cd")
                hd_new = p2.tile([128, KH, NB], F16, tag="hd")
                lstm_tail(gact, cd_new, cd, hd_new)
                cd, hd = cd_new, hd_new

                # ===== canvas += Wwrite @ h_dec ; s' = sigmoid(-canvas) =====
                for m in range(MX):
                    ps = psp.tile([128, NB], F32, tag="ps")
                    nc.tensor.matmul(ps[:], tWwr[:, 0, m, :], hd[:, 0, :],
                                     start=True, stop=False)
                    nc.tensor.matmul(ps[:], tWwr[:, 1, m, :], hd[:, 1, :],
                                     start=False, stop=True)
                    if t == 0:
                        nc.scalar.activation(tcanvas[:, m, :], ps[:], COPY)
                    else:
                        nc.vector.tensor_add(tcanvas[:, m, :], ps[:],
                                             tcanvas[:, m, :])
                # one big ACT over the whole canvas (free dim MX*NB=4096)
                if t < T - 1:
                    sprime = p2.tile([128, MX, NB], F16, tag="sp")
                    nc.scalar.activation(sprime[:], tcanvas[:], SIG, scale=-1.0)
                else:
                    canvas16 = p2.tile([128, MX, NB], F16, tag="sp")
                    nc.scalar.activation(canvas16[:], tcanvas[:], COPY)

            # ===== x_mu = Wobs @ canvas + b_obs =====
            tWobs = p1.tile([128, KX, MX, 128], F16, tag="wbig")
            nc.sync.dma_start(tWobs[:], Wobs_d[:])
            nc.sync.dma_start(tbo[:], bo_d[:])
            txmu = p1.tile([128, MX, NB], F32, tag="xbig")
            for m in range(MX):
                ps = psp.tile([128, NB], F32, tag="ps")
                for k in range(KX):
                    nc.tensor.matmul(ps[:], tWobs[:, k, m, :], canvas16[:, k, :],
                                     start=(k == 0), stop=(k == KX - 1))
                nc.vector.tensor_scalar_add(txmu[:, m, :], ps[:],
                                            tbo[:, m:m + 1])
            nc.sync.dma_start(xmu_d[:], txmu[:])
            nc.sync.dma_start(kl_d[:], tkl[:])

    nc.compile()
    return nc


def _wtiles(WT, kt, mt):
    """[K, M] -> [128, kt, mt, 128] fp16 device layout."""
    K, M = WT.shape
    assert K == kt * 128 and M == mt * 128
    return np.ascontiguousarray(
        WT.reshape(kt, 128, mt, 128).transpose(1, 0, 2, 3)).astype(np.float16)


def _bias_tiles(b, mt):
    return np.ascontiguousarray(
        np.asarray(b, np.float32).reshape(mt, 128).T)


def kernel(x, eps, W_ih_enc, b_ih_enc, W_hh_enc, b_hh_enc,
           W_ih_dec, b_ih_dec, W_hh_dec, b_hh_dec,
           W_var, b_var, W_write, b_write, W_obs, b_obs):
    global _NC, LAST_RESULT
    x = np.asarray(x, np.float32)
    eps = np.asarray(eps, np.float32)
    W_ih_enc = np.asarray(W_ih_enc, np.float32)
    W_hh_enc = np.asarray(W_hh_enc, np.float32)
    W_ih_dec = np.asarray(W_ih_dec, np.float32)
    W_hh_dec = np.asarray(W_hh_dec, np.float32)
    W_var = np.asarray(W_var, np.float32)
    W_write = np.asarray(W_write, np.float32)
    W_obs = np.asarray(W_obs, np.float32)

    W1T = W_ih_enc[:, :X].T
    W2T = W_ih_enc[:, X:2 * X].T
    W3T = np.ascontiguousarray(W_ih_enc[:, 2 * X:].T)
    w2rs = W2T.sum(axis=0)
    bsum = np.asarray(b_ih_enc, np.float32) + np.asarray(b_hh_enc, np.float32)

    common = {
        "ident": np.eye(128, dtype=np.float16),
        "WsumT": _wtiles(W1T + W2T, KX, MT),
        "W2T": _wtiles(W2T, KX, MT),
        "W3T": _wtiles(W3T, KH, MT),
        "WhhET": _wtiles(W_hh_enc.T, KH, MT),
        "WhhDT": _wtiles(W_hh_dec.T, KH, MT),
        "WwrT": _wtiles(W_write.T, KH, MX),
        "WvarT": np.ascontiguousarray(
            W_var.T.reshape(KH, 128, 2 * Z).transpose(1, 0, 2)).astype(np.float16),
        "WihdT": np.ascontiguousarray(
            W_ih_dec.T.reshape(Z, MT, 128)).astype(np.float16),
        "WobsT": _wtiles(W_obs.T, KX, MX),
        "benc": _bias_tiles(bsum - w2rs, MT),
        "benc0": _bias_tiles(bsum - 0.5 * w2rs, MT),
        "bdec": _bias_tiles(
            np.asarray(b_ih_dec, np.float32) + np.asarray(b_hh_dec, np.float32), MT),
        "bobs": _bias_tiles(np.asarray(b_obs, np.float32), MX),
    }

    in_maps = []
    for c in range(NCORES):
        sl = slice(c * NB, (c + 1) * NB)
        xT = np.ascontiguousarray(
            x[sl].T.reshape(KX, 128, NB).transpose(1, 0, 2)).astype(np.float16)
        epsT = np.ascontiguousarray(eps[:, sl, :].transpose(0, 2, 1)).astype(np.float32)
        m = dict(common)
        m["xT"] = xT
        m["epsT"] = epsT
        in_maps.append(m)

    if _NC is None:
        _NC = _build()
    res = run_bass_kernel_spmd(_NC, in_maps, core_ids=list(range(NCORES)))
    LAST_RESULT = res

    x_mu = np.empty((B, X), np.float32)
    kl = np.empty((B, Z), np.float32)
    for c in range(NCORES):
        sl = slice(c * NB, (c + 1) * NB)
        xmuT = res.results[c]["xmuT"]          # [128, MX, NB]
        x_mu[sl] = xmuT.transpose(1, 0, 2).reshape(X, NB).T
        kl[sl] = res.results[c]["klT"].T
    return x_mu, kl


# revision 17
# speedup vs baseline: 1.0598x; 1.0598x over previous
"""DRAW (nn_DRAW_17497696763951) Trainium2 Bass kernel.

Data-parallel over batch: B=4096 split across 8 NeuronCores (512 rows each),
weights replicated, T=10 recurrence local per core.

Device layout is feature-major (transposed): activations live as
[feature_partitions, batch_free] so every matmul is
  out[out_feat_tile, batch] = W_tile.T @ act_tile
with weights stationary and batch (512) as the moving free dim.

Algebra used (all precomputed on host):
  att = [x, x - sigmoid(canvas), h_dec]
  gates_enc = W1@x + W2@(x - sigmoid(canvas)) + W3@h_dec + Whh@h_enc + b
            = base + W2@sigmoid(-canvas) + W3@h_dec + Whh@h_enc
  where base = (W1+W2)@x + (b_ih + b_hh - rowsum(W2)),
  using sigmoid(-c) = 1 - sigmoid(c).
  exp(y) = 1/(1/sigmoid(y) - 1)  (keeps everything in one ACT table set)

All matmul operands are fp16 (measured ~3e-4 rel err/matmul; end-to-end
mimic ~5e-4); accumulation fp32 in PSUM; canvas/c-states/kl fp32.
"""
import numpy as np

import concourse.bacc as bacc
import concourse.tile as tile
import concourse.mybir as mybir
from concourse.bass_utils import run_bass_kernel_spmd

B, X, H, Z, T = 4096, 1024, 256, 10, 10
NCORES = 8
NB = B // NCORES       # 512 batch rows per core
KX = X // 128          # 8 contraction tiles for X-dim
KH = H // 128          # 2 contraction tiles for H-dim
MT = (4 * H) // 128    # 8 gate out tiles
MX = X // 128          # 8 X out tiles

F32 = mybir.dt.float32
F16 = mybir.dt.float16
AF = mybir.ActivationFunctionType
OP = mybir.AluOpType
SIG = AF.Sigmoid
TANH = AF.Tanh
COPY = AF.Copy

GATE_FUNC = [SIG, SIG, SIG, SIG, TANH, TANH, SIG, SIG]  # i,i,f,f,g,g,o,o

_NC = None
LAST_RESULT = None


def _build():
    nc = bacc.Bacc("TRN2", target_bir_lowering=False, debug=False)

    def din(name, shape, dt=F16):
        return nc.dram_tensor(name, shape, dt, kind="ExternalInput")

    xT_d = din("xT", (128, KX, NB))
    eps_d = din("epsT", (T, Z, NB), F32)
    Wsum_d = din("WsumT", (128, KX, MT, 128))
    W2_d = din("W2T", (128, KX, MT, 128))
    W3_d = din("W3T", (128, KH, MT, 128))
    WhhE_d = din("WhhET", (128, KH, MT, 128))
    WhhD_d = din("WhhDT", (128, KH, MT, 128))
    Wwr_d = din("WwrT", (128, KH, MX, 128))
    Wvar_d = din("WvarT", (128, KH, 2 * Z))
    Wihd_d = din("WihdT", (Z, MT, 128))
    Wobs_d = din("WobsT", (128, KX, MX, 128))
    id_d = din("ident", (128, 128))
    be_d = din("benc", (128, MT), F32)
    be0_d = din("benc0", (128, MT), F32)
    bd_d = din("bdec", (128, MT), F32)
    bo_d = din("bobs", (128, MX), F32)
    xmu_d = nc.dram_tensor("xmuT", (128, MX, NB), F32, kind="ExternalOutput")
    kl_d = nc.dram_tensor("klT", (Z, NB), F32, kind="ExternalOutput")

    with tile.TileContext(nc) as tc:
        with (
            tc.tile_pool(name="p1", bufs=1) as p1,
            tc.tile_pool(name="p2", bufs=2) as p2,
            tc.tile_pool(name="p3", bufs=3) as p3,
            tc.tile_pool(name="pg", bufs=12) as pg,
            tc.tile_pool(name="ps", bufs=8, space="PSUM") as psp,
        ):
            # ---- persistent weights ----
            tWsum = p1.tile([128, KX, MT, 128], F16, tag="wbig")  # later reused by Wobs
            tx = p1.tile([128, KX, NB], F16, tag="xbig")          # later reused by xmu
            tW2 = p1.tile([128, KX, MT, 128], F16, tag="w2")
            tW3 = p1.tile([128, KH, MT, 128], F16, tag="w3")
            tWhhE = p1.tile([128, KH, MT, 128], F16, tag="whhe")
            tWhhD = p1.tile([128, KH, MT, 128], F16, tag="whhd")
            tWwr = p1.tile([128, KH, MX, 128], F16, tag="wwr")
            tWvar = p1.tile([128, KH, 2 * Z], F16, tag="wvar")
            tWihd = p1.tile([Z, MT, 128], F16, tag="wihd")
            tbe = p1.tile([128, MT], F32, tag="be")
            tbe0 = p1.tile([128, MT], F32, tag="be0")
            tbd = p1.tile([128, MT], F32, tag="bd")
            tbo = p1.tile([128, MX], F32, tag="bo")
            tident = p1.tile([128, 128], F16, tag="ident")
            tbase = p1.tile([128, MT, NB], F16, tag="base")
            tcanvas = p1.tile([128, MX, NB], F32, tag="canvas")
            tkl = p1.tile([Z, NB], F32, tag="kl")

            # load order: base-critical, then step0-critical, then the rest
            nc.sync.dma_start(tx[:], xT_d[:])
            nc.sync.dma_start(tident[:], id_d[:])
            nc.sync.dma_start(tWsum[:], Wsum_d[:])
            nc.sync.dma_start(tbe[:], be_d[:])
            nc.sync.dma_start(tbe0[:], be0_d[:])
            nc.sync.dma_start(tWvar[:], Wvar_d[:])
            nc.sync.dma_start(tWihd[:], Wihd_d[:])
            nc.sync.dma_start(tbd[:], bd_d[:])
            nc.sync.dma_start(tWwr[:], Wwr_d[:])
            nc.sync.dma_start(tWhhD[:], WhhD_d[:])
            nc.sync.dma_start(tWhhE[:], WhhE_d[:])
            nc.sync.dma_start(tW3[:], W3_d[:])
            nc.sync.dma_start(tW2[:], W2_d[:])

            # ---- base = (W1+W2) @ x  (+ enc bias via ACT) ----
            base_ps = []
            for m in range(MT):
                ps = psp.tile([128, NB], F32, tag="ps")
                for k in range(KX):
                    nc.tensor.matmul(ps[:], tWsum[:, k, m, :], tx[:, k, :],
                                     start=(k == 0), stop=(k == KX - 1))
                nc.vector.tensor_scalar_add(tbase[:, m, :], ps[:],
                                            tbe[:, m:m + 1])
                base_ps.append(ps)

            def lstm_tail(gact, c_new, c_old, h_new):
                """c_new = sig(f)*c_old + sig(i)*tanh(g); h_new = sig(o)*tanh(c)"""
                for j in range(KH):
                    tmp = pg.tile([128, NB], F16, tag="ga")
                    nc.vector.tensor_mul(tmp[:], gact[j][:], gact[4 + j][:])
                    if c_old is None:
                        nc.vector.tensor_copy(c_new[:, j, :], tmp[:])
                    else:
                        nc.vector.tensor_mul(c_new[:, j, :], gact[2 + j][:],
                                             c_old[:, j, :])
                        nc.vector.tensor_add(c_new[:, j, :], c_new[:, j, :], tmp[:])
                    tct = pg.tile([128, NB], F16, tag="ga")
                    nc.scalar.activation(tct[:], c_new[:, j, :], TANH)
                    nc.vector.tensor_mul(h_new[:, j, :], gact[6 + j][:], tct[:])

            he = hd = ce = cd = None
            sprime = None

            for t in range(T):
                # ===== encoder gates =====
                gact = []
                if t == 0:
                    for m in range(MT):
                        g = pg.tile([128, NB], F16, tag="ga")
                        nc.scalar.activation(g[:], base_ps[m][:], GATE_FUNC[m],
                                             bias=tbe0[:, m:m + 1])
                        gact.append(g)
                else:
                    # pass 1: base (via identity matmul) + h-term matmuls for
                    # all 8 banks first (none depend on s'), then the W2 k-loop.
                    ps_list = [psp.tile([128, NB], F32, tag="ps",
                                        name=f"encps{t}_{m}")
                               for m in range(MT)]
                    for m in range(MT):
                        ps = ps_list[m]
                        nc.tensor.matmul(ps[:], tident[:], tbase[:, m, :],
                                         start=True, stop=False,
                                         skip_group_check=True)
                        nc.tensor.matmul(ps[:], tW3[:, 0, m, :], hd[:, 0, :],
                                         start=False, stop=False,
                                         skip_group_check=True)
                        nc.tensor.matmul(ps[:], tW3[:, 1, m, :], hd[:, 1, :],
                                         start=False, stop=False,
                                         skip_group_check=True)
                        nc.tensor.matmul(ps[:], tWhhE[:, 0, m, :], he[:, 0, :],
                                         start=False, stop=False,
                                         skip_group_check=True)
                        nc.tensor.matmul(ps[:], tWhhE[:, 1, m, :], he[:, 1, :],
                                         start=False, stop=False,
                                         skip_group_check=True)
                    for m in range(MT):
                        ps = ps_list[m]
                        for k in range(KX):
                            nc.tensor.matmul(ps[:], tW2[:, k, m, :], sprime[:, k, :],
                                             start=False, stop=(k == KX - 1),
                                             skip_group_check=True)

                if t > 0:
                    for m in range(MT):
                        g = pg.tile([128, NB], F16, tag="ga")
                        nc.scalar.activation(g[:], ps_list[m][:], GATE_FUNC[m])
                        gact.append(g)

                ce_new = p2.tile([128, KH, NB], F16, tag="ce")
                he_new = p2.tile([128, KH, NB], F16, tag="he")
                lstm_tail(gact, ce_new, ce, he_new)
                ce, he = ce_new, he_new

                # ===== variational head: q = Wvar @ h_enc ===== (split mu/log)
                ps_mu = psp.tile([Z, NB], F32, tag="ps")
                ps_lg = psp.tile([Z, NB], F32, tag="ps")
                for k in range(KH):
                    nc.tensor.matmul(ps_mu[:], tWvar[:, k, 0:Z], he[:, k, :],
                                     start=(k == 0), stop=(k == KH - 1))
                for k in range(KH):
                    nc.tensor.matmul(ps_lg[:], tWvar[:, k, Z:2 * Z], he[:, k, :],
                                     start=(k == 0), stop=(k == KH - 1))

                # q_std = exp(q_log) = sigmoid(q_log)/sigmoid(-q_log)
                s = p3.tile([Z, NB], F32, tag="qs")
                nc.scalar.activation(s[:], ps_lg[:], SIG)
                sm = p3.tile([Z, NB], F32, tag="qr")
                nc.scalar.activation(sm[:], ps_lg[:], SIG, scale=-1.0)
                r = p3.tile([Z, NB], F32, tag="qr")
                nc.vector.reciprocal_approx_fast(r[:], sm[:])
                qstd = p3.tile([Z, NB], F32, tag="qs")
                nc.vector.tensor_mul(qstd[:], s[:], r[:])

                teps = p2.tile([Z, NB], F32, tag="eps")
                nc.sync.dma_start(teps[:], eps_d[t])
                w = p3.tile([Z, NB], F32, tag="qr")
                nc.vector.tensor_mul(w[:], qstd[:], teps[:])
                zt = p2.tile([Z, NB], F16, tag="z")
                nc.vector.tensor_add(zt[:], w[:], ps_mu[:])

                # kl += -q_log + 0.5*(q_std^2 + q_mu^2) - 0.5
                a = p3.tile([Z, NB], F32, tag="qs")
                nc.vector.tensor_mul(a[:], qstd[:], qstd[:])
                b2 = p3.tile([Z, NB], F32, tag="qr")
                nc.scalar.activation(b2[:], ps_mu[:], AF.Square)
                nc.vector.tensor_add(a[:], a[:], b2[:])
                # a = 0.5*a - q_log
                nc.vector.scalar_tensor_tensor(a[:], a[:], 0.5, ps_lg[:],
                                               op0=OP.mult, op1=OP.subtract)
                if t == 0:
                    nc.vector.tensor_scalar_add(tkl[:], a[:], -0.5)
                else:
                    nc.vector.scalar_tensor_tensor(tkl[:], a[:], 0.5, tkl[:],
                                                   op0=OP.subtract, op1=OP.add)

                # ===== decoder gates =====
                gact = []
                for m in range(MT):
                    ps = psp.tile([128, NB], F32, tag="ps", name=f"decps{t}_{m}")
                    if t > 0:
                        nc.tensor.matmul(ps[:], tWhhD[:, 0, m, :], hd[:, 0, :],
                                         start=True, stop=False,
                                         skip_group_check=True)
                        nc.tensor.matmul(ps[:], tWhhD[:, 1, m, :], hd[:, 1, :],
                                         start=False, stop=False,
                                         skip_group_check=True)
                    nc.tensor.matmul(ps[:], tWihd[:, m, :], zt[:],
                                     start=(t == 0), stop=True,
                                     skip_group_check=True)
                    g = pg.tile([128, NB], F16, tag="ga")
                    nc.scalar.activation(g[:], ps[:], GATE_FUNC[m],
                                         bias=tbd[:, m:m + 1])
                    gact.append(g)

                cd_new = p2.tile([128, KH, NB], F16, tag="cd")
                hd_new = p2.tile([128, KH, NB], F16, tag="hd")
                lstm_tail(gact, cd_new, cd, hd_new)
                cd, hd = cd_new, hd_new

                # ===== canvas += Wwrite @ h_dec ; s' = sigmoid(-canvas) =====
                for m in range(MX):
                    ps = psp.tile([128, NB], F32, tag="ps")
                    nc.tensor.matmul(ps[:], tWwr[:, 0, m, :], hd[:, 0, :],
                                     start=True, stop=False)
                    nc.tensor.matmul(ps[:], tWwr[:, 1, m, :], hd[:, 1, :],
                                     start=False, stop=True)
                    if t == 0:
                        nc.scalar.activation(tcanvas[:, m, :], ps[:], COPY)
                    else:
                        nc.vector.tensor_add(tcanvas[:, m, :], ps[:],
                                             tcanvas[:, m, :])
                # one big ACT over the whole canvas (free dim MX*NB=4096)
                if t < T - 1:
                    sprime = p2.tile([128, MX, NB], F16, tag="sp")
                    nc.scalar.activation(sprime[:], tcanvas[:], SIG, scale=-1.0)
                else:
                    canvas16 = p2.tile([128, MX, NB], F16, tag="sp")
                    nc.scalar.activation(canvas16[:], tcanvas[:], COPY)

            # ===== x_mu = Wobs @ canvas + b_obs =====
            tWobs = p1.tile([128, KX, MX, 128], F16, tag="wbig")
            nc.sync.dma_start(tWobs[:], Wobs_d[:])
            nc.sync.dma_start(tbo[:], bo_d[:])
            txmu = p1.tile([128, MX, NB], F32, tag="xbig")
            for m in range(MX):
                ps = psp.tile([128, NB], F32, tag="ps")
                for k in range(KX):
                    nc.tensor.matmul(ps[:], tWobs[:, k, m, :], canvas16[:, k, :],
                                     start=(k == 0), stop=(k == KX - 1))
                nc.vector.tensor_scalar_add(txmu[:, m, :], ps[:],
                                            tbo[:, m:m + 1])
            nc.sync.dma_start(xmu_d[:], txmu[:])
            nc.sync.dma_start(kl_d[:], tkl[:])

    nc.compile()
    return nc


def _wtiles(WT, kt, mt):
    """[K, M] -> [128, kt, mt, 128] fp16 device layout."""
    K, M = WT.shape
    assert K == kt * 128 and M == mt * 128
    return np.ascontiguousarray(
        WT.reshape(kt, 128, mt, 128).transpose(1, 0, 2, 3)).astype(np.float16)


def _bias_tiles(b, mt):
    return np.ascontiguousarray(
        np.asarray(b, np.float32).reshape(mt, 128).T)


def kernel(x, eps, W_ih_enc, b_ih_enc, W_hh_enc, b_hh_enc,
           W_ih_dec, b_ih_dec, W_hh_dec, b_hh_dec,
           W_var, b_var, W_write, b_write, W_obs, b_obs):
    global _NC, LAST_RESULT
    x = np.asarray(x, np.float32)
    eps = np.asarray(eps, np.float32)
    W_ih_enc = np.asarray(W_ih_enc, np.float32)
    W_hh_enc = np.asarray(W_hh_enc, np.float32)
    W_ih_dec = np.asarray(W_ih_dec, np.float32)
    W_hh_dec = np.asarray(W_hh_dec, np.float32)
    W_var = np.asarray(W_var, np.float32)
    W_write = np.asarray(W_write, np.float32)
    W_obs = np.asarray(W_obs, np.float32)

    W1T = W_ih_enc[:, :X].T
    W2T = W_ih_enc[:, X:2 * X].T
    W3T = np.ascontiguousarray(W_ih_enc[:, 2 * X:].T)
    w2rs = W2T.sum(axis=0)
    bsum = np.asarray(b_ih_enc, np.float32) + np.asarray(b_hh_enc, np.float32)

    common = {
        "ident": np.eye(128, dtype=np.float16),
        "WsumT": _wtiles(W1T + W2T, KX, MT),
        "W2T": _wtiles(W2T, KX, MT),
        "W3T": _wtiles(W3T, KH, MT),
        "WhhET": _wtiles(W_hh_enc.T, KH, MT),
        "WhhDT": _wtiles(W_hh_dec.T, KH, MT),
        "WwrT": _wtiles(W_write.T, KH, MX),
        "WvarT": np.ascontiguousarray(
            W_var.T.reshape(KH, 128, 2 * Z).transpose(1, 0, 2)).astype(np.float16),
        "WihdT": np.ascontiguousarray(
            W_ih_dec.T.reshape(Z, MT, 128)).astype(np.float16),
        "WobsT": _wtiles(W_obs.T, KX, MX),
        "benc": _bias_tiles(bsum - w2rs, MT),
        "benc0": _bias_tiles(bsum - 0.5 * w2rs, MT),
        "bdec": _bias_tiles(
            np.asarray(b_ih_dec, np.float32) + np.asarray(b_hh_dec, np.float32), MT),
        "bobs": _bias_tiles(np.asarray(b_obs, np.float32), MX),
    }

    in_maps = []
    for c in range(NCORES):
        sl = slice(c * NB, (c + 1) * NB)
        xT = np.ascontiguousarray(
            x[sl].T.reshape(KX, 128, NB).transpose(1, 0, 2)).astype(np.float16)
        epsT = np.ascontiguousarray(eps[:, sl, :].transpose(0, 2, 1)).astype(np.float32)
        m = dict(common)
        m["xT"] = xT
        m["epsT"] = epsT
        in_maps.append(m)

    if _NC is None:
        _NC = _build()
    res = run_bass_kernel_spmd(_NC, in_maps, core_ids=list(range(NCORES)))
    LAST_RESULT = res

    x_mu = np.empty((B, X), np.float32)
    kl = np.empty((B, Z), np.float32)
    for c in range(NCORES):
        sl = slice(c * NB, (c + 1) * NB)
        xmuT = res.results[c]["xmuT"]          # [128, MX, NB]
        x_mu[sl] = xmuT.transpose(1, 0, 2).reshape(X, NB).T
        kl[sl] = res.results[c]["klT"].T
    return x_mu, kl
